# revision 24
# baseline (speedup 1.0000x reference)
"""Trainium2 Bass kernel for nn_BinaryDiff.

Reference computes:
    out = x @ base_T + coeff * (x @ signs),  signs = 2*mask_bits - 1
which algebraically equals a single dense matmul:
    out = x @ W,  W = base_T + coeff * (2*mask_bits - 1)

Strategy:
  - Fold W on host (cheap weight preprocessing, 16.8M elements).
  - Shard rows of x (M = B*S = 8192) across 8 cores: 1024 rows each.
    W is replicated. Per-core HBM traffic ~100MB, PE work ~34.4 GFLOP.
  - Host passes x pre-transposed (K-major) so both matmul operands load
    with K on partitions, no on-device transpose needed.
  - Matmul runs in float32r (FP22 multiply, FP32 accumulate): 1 PE
    cycle/row at N=512 vs 4 for full fp32.
  - Device loop: resident xT panel [4096 x 1024] in SBUF; for each of 8
    N-panels of 512 cols, accumulate 32 K-tiles into 8 PSUM banks (one
    per M-tile), drain via DVE copy, DMA out.
"""

import numpy as np

B, S, DIN, DOUT = 4, 2048, 4096, 4096
NCORES = 8
MTOT = B * S
MSHARD = MTOT // NCORES  # 1024

_CACHE = {}


def _build(din=DIN, dout=DOUT, mshard=MSHARD):
    import concourse.bacc as bacc
    import concourse.mybir as mybir
    import concourse.tile as tile

    f32 = mybir.dt.float32
    f32r = mybir.dt.float32r

    KT = din // 128     # K tiles (contraction)
    MT = mshard // 128  # M tiles -> PSUM banks
    NP = dout // 512    # N panels
    assert MT <= 8, "PSUM has 8 banks"

    nc = bacc.Bacc()
    xt = nc.declare_dram_parameter("xt", [din, mshard], f32r, isOutput=False)
    w = nc.declare_dram_parameter("w", [din, dout], f32r, isOutput=False)
    out = nc.declare_dram_parameter("out", [mshard, dout], f32, isOutput=True)

    with tile.TileContext(nc) as tc:
        with (
            tc.tile_pool(name="xt_pool", bufs=1) as xt_pool,
            tc.tile_pool(name="w_pool", bufs=8) as w_pool,
            tc.tile_pool(name="ps_pool", bufs=1, space="PSUM") as ps_pool,
            tc.tile_pool(name="o_pool", bufs=16) as o_pool,
        ):
            # Resident x^T panel as one SBUF tile per k-tile so dependency
            # tracking is per-k: panel-0 matmuls start as soon as their own
            # k-tile has landed (loads interleaved with the k-loop below).
            xts = [
                xt_pool.tile([128, mshard], f32r, tag=f"xt{k}", name=f"xt{k}")
                for k in range(KT)
            ]

            for p in range(NP):
                n0 = p * 512
                last_panel = p == NP - 1
                ps = [
                    ps_pool.tile([128, 512], f32, tag=f"ps{m}", name=f"ps{m}_{p}")
                    for m in range(MT)
                ]
                for k in range(KT):
                    if p == 0:
                        if k == 0:
                            # Split the first k-tile's load: the very first
                            # matmuls need only the low m columns, and Tile
                            # tracks subtile ranges, so compute starts as soon
                            # as the small piece lands.
                            cut = 256 if mshard > 256 else mshard // 2
                            nc.sync.dma_start(xts[0][:, :cut], xt[:128, :cut])
                            nc.sync.dma_start(xts[0][:, cut:], xt[:128, cut:])
                        elif k < 2:
                            # First k-tiles on the HWDGE sync ring: ~0.6us
                            # first byte vs ~2us SWDGE, so compute starts fast.
                            nc.sync.dma_start(xts[k][:], xt[k * 128:(k + 1) * 128, :])
                        else:
                            # Rest split across the gpsimd (SWDGE) and scalar
                            # (ACT HWDGE) rings — the scalar ring is idle until
                            # panel 0's stores begin, and the ramp is
                            # supply-bound, so two issue rings feed the PE
                            # faster than one.
                            xt_eng = nc.gpsimd if k % 2 == 0 else nc.scalar
                            xt_eng.dma_start(xts[k][:], xt[k * 128:(k + 1) * 128, :])
                    w_t = w_pool.tile([128, 512], f32r, tag="w", name=f"w_{p}_{k}")
                    w_eng = nc.scalar if (p == 0 and k == 0) else nc.sync
                    w_eng.dma_start(w_t[:], w[k * 128:(k + 1) * 128, n0:n0 + 512])
                    for m in range(MT):
                        nc.tensor.matmul(
                            ps[m][:],
                            xts[k][:, m * 128:(m + 1) * 128],
                            w_t[:],
                            start=(k == 0),
                            stop=(k == KT - 1),
                        )
                for m in range(MT):
                    o_t = o_pool.tile([128, 512], f32, tag="o", name=f"o_{p}_{m}")
                    if last_panel and (m % 2 == 1):
                        # Split the final drain across ACT and DVE so the
                        # serial copy chain after the last matmul halves.
                        nc.scalar.copy(o_t[:], ps[m][:])
                    else:
                        nc.vector.tensor_copy(o_t[:], ps[m][:])
                    # Stores ride the scalar (ACT) HWDGE ring so the next
                    # panel's W prefetch isn't queued behind them; the last
                    # panel's stores go to the (now idle) sync ring to keep
                    # ACT free for its share of the drain copies.
                    st_eng = nc.sync if last_panel else nc.scalar
                    st_eng.dma_start(out[m * 128:(m + 1) * 128, n0:n0 + 512], o_t[:])

    nc.finalize()
    return nc


def _get_nc():
    if "nc" not in _CACHE:
        _CACHE["nc"] = _build()
    return _CACHE["nc"]


def _run(x, base_T, mask_bits, coeff, trace=False):
    from concourse.bass_utils import run_bass_kernel_spmd

    nc = _get_nc()

    W = (np.asarray(base_T, dtype=np.float32)
         + np.float32(coeff[0]) * (2.0 * np.asarray(mask_bits, dtype=np.float32) - 1.0))
    W = np.ascontiguousarray(W, dtype=np.float32)
    X = np.asarray(x, dtype=np.float32).reshape(MTOT, DIN)

    in_maps = []
    for c in range(NCORES):
        xt_c = np.ascontiguousarray(X[c * MSHARD:(c + 1) * MSHARD, :].T)
        in_maps.append({"xt": xt_c, "w": W})

    res = run_bass_kernel_spmd(nc, in_maps, list(range(NCORES)), trace=trace)
    outs = [res.results[c]["out"] for c in range(NCORES)]
    full = np.concatenate(outs, axis=0).reshape(B, S, DOUT).astype(np.float32)
    return full, res


def kernel(x, base_T, mask_bits, coeff):
    full, _ = _run(x, base_T, mask_bits, coeff, trace=False)
    return full


# revision 25
# speedup vs baseline: 1.0161x; 1.0161x over previous
"""Trainium2 Bass kernel for nn_BinaryDiff.

Reference computes:
    out = x @ base_T + coeff * (x @ signs),  signs = 2*mask_bits - 1
which algebraically equals a single dense matmul:
    out = x @ W,  W = base_T + coeff * (2*mask_bits - 1)

Strategy:
  - Fold W on host (cheap weight preprocessing, 16.8M elements).
  - Shard rows of x (M = B*S = 8192) across 8 cores: 1024 rows each.
    W is replicated. Per-core HBM traffic ~100MB, PE work ~34.4 GFLOP.
  - Host passes x pre-transposed (K-major) so both matmul operands load
    with K on partitions, no on-device transpose needed.
  - Matmul runs in float32r (FP22 multiply, FP32 accumulate): 1 PE
    cycle/row at N=512 vs 4 for full fp32.
  - Device loop: resident xT panel [4096 x 1024] in SBUF; for each of 8
    N-panels of 512 cols, accumulate 32 K-tiles into 8 PSUM banks (one
    per M-tile), drain via DVE copy, DMA out.
"""

import numpy as np

B, S, DIN, DOUT = 4, 2048, 4096, 4096
NCORES = 8
MTOT = B * S
MSHARD = MTOT // NCORES  # 1024

_CACHE = {}


def _build(din=DIN, dout=DOUT, mshard=MSHARD):
    import concourse.bacc as bacc
    import concourse.mybir as mybir
    import concourse.tile as tile

    f32 = mybir.dt.float32
    f32r = mybir.dt.float32r

    KT = din // 128     # K tiles (contraction)
    MT = mshard // 128  # M tiles -> PSUM banks
    NP = dout // 512    # N panels
    assert MT <= 8, "PSUM has 8 banks"

    nc = bacc.Bacc()
    xt = nc.declare_dram_parameter("xt", [din, mshard], f32r, isOutput=False)
    w = nc.declare_dram_parameter("w", [din, dout], f32r, isOutput=False)
    out = nc.declare_dram_parameter("out", [mshard, dout], f32, isOutput=True)

    with tile.TileContext(nc) as tc:
        with (
            tc.tile_pool(name="xt_pool", bufs=1) as xt_pool,
            tc.tile_pool(name="w_pool", bufs=8) as w_pool,
            tc.tile_pool(name="ps_pool", bufs=1, space="PSUM") as ps_pool,
            tc.tile_pool(name="o_pool", bufs=16) as o_pool,
        ):
            # Resident x^T panel as one SBUF tile per k-tile so dependency
            # tracking is per-k: panel-0 matmuls start as soon as their own
            # k-tile has landed (loads interleaved with the k-loop below).
            xts = [
                xt_pool.tile([128, mshard], f32r, tag=f"xt{k}", name=f"xt{k}")
                for k in range(KT)
            ]

            for p in range(NP):
                n0 = p * 512
                last_panel = p == NP - 1
                ps = [
                    ps_pool.tile([128, 512], f32, tag=f"ps{m}", name=f"ps{m}_{p}")
                    for m in range(MT)
                ]
                for k in range(KT):
                    if p == 0:
                        if k == 0:
                            # Split the first k-tile's load: the very first
                            # matmuls need only the low m columns, and Tile
                            # tracks subtile ranges, so compute starts as soon
                            # as the small piece lands.
                            cut = 256 if mshard > 256 else mshard // 2
                            nc.sync.dma_start(xts[0][:, :cut], xt[:128, :cut])
                            nc.sync.dma_start(xts[0][:, cut:], xt[:128, cut:])
                        elif k < 2:
                            # First k-tiles on the HWDGE sync ring: ~0.6us
                            # first byte vs ~2us SWDGE, so compute starts fast.
                            nc.sync.dma_start(xts[k][:], xt[k * 128:(k + 1) * 128, :])
                        else:
                            # Rest ride the gpsimd (SWDGE) ring so they don't
                            # contend with the W-tile FIFO on the sync ring.
                            nc.gpsimd.dma_start(xts[k][:], xt[k * 128:(k + 1) * 128, :])
                    w_t = w_pool.tile([128, 512], f32r, tag="w", name=f"w_{p}_{k}")
                    w_eng = nc.scalar if (p == 0 and k == 0) else nc.sync
                    w_eng.dma_start(w_t[:], w[k * 128:(k + 1) * 128, n0:n0 + 512])
                    for m in range(MT):
                        nc.tensor.matmul(
                            ps[m][:],
                            xts[k][:, m * 128:(m + 1) * 128],
                            w_t[:],
                            start=(k == 0),
                            stop=(k == KT - 1),
                        )
                for m in range(MT):
                    o_t = o_pool.tile([128, 512], f32, tag="o", name=f"o_{p}_{m}")
                    if last_panel and (m % 2 == 1):
                        # Split the final drain across ACT and DVE so the
                        # serial copy chain after the last matmul halves.
                        nc.scalar.copy(o_t[:], ps[m][:])
                    else:
                        nc.vector.tensor_copy(o_t[:], ps[m][:])
                    # Stores ride the scalar (ACT) HWDGE ring so the next
                    # panel's W prefetch isn't queued behind them; the last
                    # panel's stores go to the (now idle) sync ring to keep
                    # ACT free for its share of the drain copies.
                    st_eng = nc.sync if last_panel else nc.scalar
                    st_eng.dma_start(out[m * 128:(m + 1) * 128, n0:n0 + 512], o_t[:])

    nc.finalize()
    return nc


def _get_nc():
    if "nc" not in _CACHE:
        _CACHE["nc"] = _build()
    return _CACHE["nc"]


def _run(x, base_T, mask_bits, coeff, trace=False):
    from concourse.bass_utils import run_bass_kernel_spmd

    nc = _get_nc()

    W = (np.asarray(base_T, dtype=np.float32)
         + np.float32(coeff[0]) * (2.0 * np.asarray(mask_bits, dtype=np.float32) - 1.0))
    W = np.ascontiguousarray(W, dtype=np.float32)
    X = np.asarray(x, dtype=np.float32).reshape(MTOT, DIN)

    in_maps = []
    for c in range(NCORES):
        xt_c = np.ascontiguousarray(X[c * MSHARD:(c + 1) * MSHARD, :].T)
        in_maps.append({"xt": xt_c, "w": W})

    res = run_bass_kernel_spmd(nc, in_maps, list(range(NCORES)), trace=trace)
    outs = [res.results[c]["out"] for c in range(NCORES)]
    full = np.concatenate(outs, axis=0).reshape(B, S, DOUT).astype(np.float32)
    return full, res


def kernel(x, base_T, mask_bits, coeff):
    full, _ = _run(x, base_T, mask_bits, coeff, trace=False)
    return full
